# revision 3
# baseline (speedup 1.0000x reference)
"""Trainium2 Bass kernel for MultiHeadLatentAttention (MLA) forward — v2.

Sharding: 8 cores = 2 (batch) x 4 (head-groups of 4 heads), as baseline.
v2 changes vs baseline:
  * all matmul operands in bf16 (PE rate unchanged at 1 cycle/row, DMA and
    SBUF footprint halved; fp32 accumulation in PSUM throughout)
  * Q^T / K^T / V / k_rope stay SBUF-resident between the projection and
    attention phases — no DRAM scratch spill/reload
  * rms-norm weights folded into the up-projection weights on the host;
    the norm scale broadcast and the softmax reciprocal broadcast both go
    through tiny ones-matmuls on the PE instead of DRAM round-trips
  * attention runs qt-outer / head-inner and the head-sliced o_proj for
    each 512-token block is emitted right after it, so the output DMA
    overlaps the remaining attention compute
"""
import numpy as np

import concourse.bass as bass
import concourse.tile as tile
from concourse import bacc
from concourse import mybir
from concourse.bass_utils import run_bass_kernel_spmd

# ---- problem dims (hardcoded) ----
B, S, HID = 2, 2048, 2048
NH = 16
QLORA, KVLORA = 1024, 512
NOPE, ROPE, VD = 128, 64, 128
QK = NOPE + ROPE
EPS = 1e-6
NHL = 4               # heads per core
P = 128
FD = 512              # matmul moving/free dim
SCALE = 1.0 / float(np.sqrt(np.float32(QK)))

F32 = mybir.dt.float32
F32R = mybir.dt.float32r
BF16 = mybir.dt.bfloat16
NPBF16 = mybir.dt.np(BF16)
AF = mybir.ActivationFunctionType


def _r(ap):
    return ap.bitcast(F32R)


def build_nc(s=S):
    sq = min(FD, s)
    n_sq = s // sq         # 4 s-quarters for projections
    n_kc = s // P          # 16 attention k chunks
    n_qt = s // FD         # 4 attention q tiles
    nc = bacc.Bacc()

    xT = nc.dram_tensor("xT", [HID, s], BF16, kind="ExternalInput")
    w_down = nc.dram_tensor("w_down", [HID, 1664], BF16, kind="ExternalInput")
    w_qu = nc.dram_tensor("w_qu", [QLORA, 768], BF16, kind="ExternalInput")
    w_kvu_kn = nc.dram_tensor("w_kvu_kn", [KVLORA, 512], BF16, kind="ExternalInput")
    w_kvu_v = nc.dram_tensor("w_kvu_v", [KVLORA, 512], BF16, kind="ExternalInput")
    w_o = nc.dram_tensor("w_o", [512, HID], BF16, kind="ExternalInput")
    cos2_d = nc.dram_tensor("cos2", [P, s], F32, kind="ExternalInput")
    sinp_d = nc.dram_tensor("sin_pre", [P, s], F32, kind="ExternalInput")
    mask_d = nc.dram_tensor("mask_p", [P, n_kc], F32, kind="ExternalInput")
    onesc_d = nc.dram_tensor("ones_col", [P, 1], BF16, kind="ExternalInput")
    onesr_d = nc.dram_tensor("ones_row", [1, P], F32, kind="ExternalInput")

    out_d = nc.dram_tensor("out", [s, HID], F32, kind="ExternalOutput")

    w_down3 = w_down[:, :].rearrange("(hk p) c -> p hk c", p=P)
    xT3 = xT[:, :].rearrange("(hk p) t -> p hk t", p=P)
    w_qu3 = w_qu[:, :].rearrange("(kk p) c -> p kk c", p=P)

    with tile.TileContext(nc) as tc:
      with tc.tile_pool(name="consts", bufs=1) as consts:
        mask_sb = consts.tile([P, n_kc], F32)
        nc.sync.dma_start(mask_sb, mask_d[:, :])
        ones_col = consts.tile([P, 1], BF16)
        nc.sync.dma_start(ones_col, onesc_d[:, :])
        ones_row = consts.tile([1, P], F32R)
        nc.sync.dma_start(ones_row, onesr_d[:, :].bitcast(F32R))
        cos2 = consts.tile([P, s], F32)
        sinp = consts.tile([P, s], F32)
        # SBUF-resident attention operands (written in P, read in A).
        # QT: 4 nope blocks + 2 packed rope blocks (heads 0|1 and 2|3 in
        # partition halves).  KR holds k_rope duplicated in both halves
        # (the host duplicates the rope columns of w_kv_down).
        QT = consts.tile([P, 6, s], BF16)
        KN = consts.tile([P, 4, s], BF16)
        KR = consts.tile([P, s], BF16)
        VT = consts.tile([P, n_kc, 512], BF16)

        # ---------------- P: projections, per s-quarter ----------------
        with tc.tile_pool(name="pw", bufs=1) as pw, \
             tc.tile_pool(name="pwd", bufs=2) as pwd, \
             tc.tile_pool(name="px", bufs=2) as px, \
             tc.tile_pool(name="plat", bufs=2) as plat, \
             tc.tile_pool(name="ptmp", bufs=2) as ptmp, \
             tc.tile_pool(name="pst", bufs=1) as pst, \
             tc.tile_pool(name="pscl", bufs=2) as pscl, \
             tc.tile_pool(name="ps_mm", bufs=3, space="PSUM") as ps_mm, \
             tc.tile_pool(name="ps_sum", bufs=2, space="PSUM") as ps_sum, \
             tc.tile_pool(name="ps_scl", bufs=1, space="PSUM") as ps_scl:
            wqu = pw.tile([P, 8, 768], BF16)
            wkn = pw.tile([P, 4, 512], BF16)
            wv = pw.tile([P, 4, 512], BF16)
            # deferred loads, staggered into quarter 0's rt loop so the first
            # down-proj matmuls aren't gated on these big transfers
            _pw_loads = [
                lambda: nc.sync.dma_start(cos2, cos2_d[:, :]),
                lambda: nc.sync.dma_start(sinp, sinp_d[:, :]),
                lambda: nc.sync.dma_start(
                    wkn, w_kvu_kn[:, :].rearrange("(kk p) c -> p kk c", p=P)),
                lambda: nc.sync.dma_start(
                    wv, w_kvu_v[:, :].rearrange("(kk p) c -> p kk c", p=P)),
                lambda: nc.sync.dma_start(wqu[:, :, 0:256], w_qu3[:, :, 0:256]),
                lambda: nc.sync.dma_start(wqu[:, :, 256:512], w_qu3[:, :, 256:512]),
                lambda: nc.sync.dma_start(wqu[:, :, 512:768], w_qu3[:, :, 512:768]),
            ]

            def rope_from(src, out_t, sl):
                """out = x*cos2 + blockswap(x*sin_pre) on two independent
                64-row halves (packed rope of two heads, or duplicated
                k_rope).

                DVE ops can't read across partitions, so the 32-row block
                swaps go through SBUF->SBUF DMAs into tsw_t.
                """
                tmp_t = ptmp.tile([P, sq], F32, tag="ropetmp")
                tsw_t = ptmp.tile([P, sq], F32, tag="ropesw")
                nc.vector.tensor_mul(tmp_t, src, sinp[:, sl])
                nc.vector.tensor_mul(out_t, src, cos2[:, sl])
                nc.sync.dma_start(tsw_t[0:32], tmp_t[32:64])
                nc.sync.dma_start(tsw_t[32:64], tmp_t[0:32])
                nc.sync.dma_start(tsw_t[64:96], tmp_t[96:128])
                nc.sync.dma_start(tsw_t[96:128], tmp_t[64:96])
                nc.vector.tensor_add(out_t, out_t, tsw_t)

            _xqs = {}

            def get_xq(q):
                if q not in _xqs:
                    xq = px.tile([P, 16, sq], BF16, tag="xq")
                    nc.sync.dma_start(xq, xT3[:, :, q * sq:(q + 1) * sq])
                    _xqs[q] = xq
                return _xqs[q]

            def emit_down_stats(q):
                """Down-proj + rms stats + norm muls for quarter q.

                The sum-of-squares matmul for block rt is emitted after the
                down matmuls of block rt+1 so the PE never head-of-line
                waits on the ACT Square latency.
                """
                sl = slice(q * sq, (q + 1) * sq)
                xq = get_xq(q)
                if q + 1 < n_sq:
                    get_xq(q + 1)  # prefetch next quarter behind this one
                lat = plat.tile([P, 12, sq], BF16, tag="lat")
                sumq_ps = ps_sum.tile([1, sq], F32, tag="sumq")
                sumkv_ps = ps_sum.tile([1, sq], F32, tag="sumkv")
                pend = None
                for rt in range(13):
                    wd_t = pwd.tile([P, 16, P], BF16, tag="wd")
                    nc.sync.dma_start(wd_t, w_down3[:, :, rt * P:(rt + 1) * P])
                    if q == 0 and 2 <= rt < 2 + len(_pw_loads):
                        _pw_loads[rt - 2]()
                    ps = ps_mm.tile([P, sq], F32, tag="dps")
                    for hk in range(16):
                        nc.tensor.matmul(
                            ps, wd_t[:, hk, :], xq[:, hk, :],
                            start=(hk == 0), stop=(hk == 15))
                    if pend is not None:
                        nc.tensor.matmul(*pend)
                        pend = None
                    if rt < 12:
                        sq_t = ptmp.tile([P, sq], BF16, tag="sq")
                        nc.scalar.activation(sq_t, ps, AF.Square)
                        if rt % 2 == 0:
                            nc.vector.tensor_copy(lat[:, rt, :], ps)
                        else:
                            nc.scalar.copy(lat[:, rt, :], ps)
                        tgt = sumq_ps if rt < 8 else sumkv_ps
                        pend = (tgt, ones_col, sq_t,
                                rt in (0, 8), rt in (7, 11))
                    else:
                        rope_from(ps, KR[:, sl], sl)
                # -- rmsnorm scales: rsqrt(sum/n + eps), Newton-refined,
                #    broadcast to 128 partitions via ones-matmul --
                scls = []
                for nm, sums, n_el in (("q", sumq_ps, QLORA), ("kv", sumkv_ps, KVLORA)):
                    m_t = pst.tile([1, sq], F32, tag=f"m{nm}", name=f"m{nm}")
                    nc.vector.tensor_scalar(
                        m_t, sums, 1.0 / n_el, EPS,
                        op0=mybir.AluOpType.mult, op1=mybir.AluOpType.add)
                    inv_t = pst.tile([1, sq], F32, tag=f"inv{nm}", name=f"inv{nm}")
                    nc.vector.reciprocal(inv_t, m_t)
                    y_t = pst.tile([1, sq], F32R, tag=f"y{nm}", name=f"y{nm}")
                    nc.scalar.activation(y_t, inv_t, AF.Sqrt)
                    t_t = pst.tile([1, sq], F32, tag=f"t{nm}", name=f"t{nm}")
                    nc.vector.tensor_mul(t_t, y_t, y_t)
                    nc.vector.tensor_mul(t_t, t_t, m_t)
                    nc.vector.tensor_scalar(
                        t_t, t_t, -0.5, 1.5,
                        op0=mybir.AluOpType.mult, op1=mybir.AluOpType.add)
                    nc.vector.tensor_mul(y_t, y_t, t_t)
                    scl_ps = ps_scl.tile([P, sq], F32, tag="scl")
                    nc.tensor.matmul(scl_ps, ones_row, y_t, start=True, stop=True)
                    scl = pscl.tile([P, sq], F32, tag=f"scl{nm}")
                    nc.scalar.copy(scl, scl_ps)
                    scls.append(scl)
                for rt in range(12):
                    rs = scls[0] if rt < 8 else scls[1]
                    nc.vector.tensor_mul(lat[:, rt, :], lat[:, rt, :], rs)
                return lat

            def emit_up(q, lat):
                """Up projections for quarter q (emitted one quarter late so
                the PE chews quarter q+1's down-proj while q's norm muls
                drain on the DVE)."""
                sl = slice(q * sq, (q + 1) * sq)
                for ro in range(6):
                    ps = ps_mm.tile([P, sq], F32, tag="dps")
                    for kk in range(8):
                        nc.tensor.matmul(
                            ps, wqu[:, kk, ro * P:(ro + 1) * P], lat[:, kk, :],
                            start=(kk == 0), stop=(kk == 7))
                    if ro < 4:
                        nc.any.tensor_copy(QT[:, ro, sl], ps)
                    else:
                        rope_from(ps, QT[:, ro, sl], sl)
                for i in range(4):
                    ps = ps_mm.tile([P, sq], F32, tag="dps")
                    for kk in range(4):
                        nc.tensor.matmul(
                            ps, wkn[:, kk, i * P:(i + 1) * P], lat[:, 8 + kk, :],
                            start=(kk == 0), stop=(kk == 3))
                    nc.any.tensor_copy(KN[:, i, sl], ps)
                for sc in range(sq // P):
                    ps = ps_mm.tile([P, 512], F32, tag="dps")
                    for kk in range(4):
                        nc.tensor.matmul(
                            ps, lat[:, 8 + kk, sc * P:(sc + 1) * P], wv[:, kk, :],
                            start=(kk == 0), stop=(kk == 3))
                    nc.any.tensor_copy(VT[:, q * (sq // P) + sc, :], ps)

            prev = None
            for q in range(n_sq):
                lat = emit_down_stats(q)
                if prev is not None:
                    emit_up(*prev)
                prev = (q, lat)
            emit_up(*prev)

        # ---------------- A + O: attention interleaved with o_proj ----------------
        with tc.tile_pool(name="aw", bufs=1) as aw, \
             tc.tile_pool(name="aat", bufs=2) as aat, \
             tc.tile_pool(name="aexp", bufs=8) as aexp, \
             tc.tile_pool(name="aacc", bufs=2) as aacc, \
             tc.tile_pool(name="asm", bufs=2) as asm, \
             tc.tile_pool(name="oo", bufs=3) as oo, \
             tc.tile_pool(name="ps_s", bufs=2, space="PSUM") as ps_s, \
             tc.tile_pool(name="ps_pv", bufs=2, space="PSUM") as ps_pv, \
             tc.tile_pool(name="ps_ss", bufs=2, space="PSUM") as ps_ss, \
             tc.tile_pool(name="ps_o", bufs=2, space="PSUM") as ps_o:
            wo_sb = aw.tile([P, 4, HID], BF16)
            nc.sync.dma_start(wo_sb, w_o[:, :].rearrange("(c p) n -> p c n", p=P))
            for qt in range(n_qt):
                qsl = slice(qt * FD, (qt + 1) * FD)
                attnT = aat.tile([P, 4, FD], BF16, tag="attnT")
                for h in range(4):
                    pv_ps = ps_pv.tile([P, FD], F32, tag="pv")
                    ssum_ps = ps_ss.tile([1, FD], F32, tag="ss")
                    r0 = 64 * (h % 2)
                    rsl = slice(r0, r0 + 64)
                    # software-pipelined: PV for chunk kc is emitted two
                    # chunks late so the PE never waits on the ACT Exp
                    # latency.  The softmax denominator sums groups of 4 e
                    # tiles on the DVE first, so only 4 ones-matmuls hit the
                    # PE per (head, q-tile).
                    es = []
                    pairs = {}
                    ssum_pend = []
                    for kc in range(n_kc + 4):
                        if kc < n_kc:
                            ks = slice(kc * P, (kc + 1) * P)
                            s_ps = ps_s.tile([P, FD], F32, tag="s")
                            nc.tensor.matmul(s_ps, KN[:, h, ks], QT[:, h, qsl],
                                             start=True, stop=False)
                            nc.tensor.matmul(s_ps, KR[rsl, ks],
                                             QT[rsl, 4 + h // 2, qsl],
                                             start=False, stop=True)
                            e_t = aexp.tile([P, FD], BF16, tag="e")
                            nc.scalar.activation(e_t, s_ps, AF.Exp,
                                                 bias=mask_sb[:, kc:kc + 1],
                                                 scale=SCALE)
                            es.append(e_t)
                        if 2 <= kc < n_kc + 2:
                            kd = kc - 2
                            nc.tensor.matmul(pv_ps, VT[:, kd, h * P:(h + 1) * P],
                                             es[kd],
                                             start=(kd == 0), stop=(kd == n_kc - 1))
                            # denominator: DVE pair-adds emitted as soon as e
                            # tiles land; the group ones-matmul runs 2 chunks
                            # later so the PE never waits on the add tree
                            if kd % 2 == 1:
                                pr = aacc.tile([P, FD], BF16, tag=f"pr{(kd // 2) % 2}")
                                nc.vector.tensor_add(pr, es[kd - 1], es[kd])
                                pairs[kd // 2] = pr
                            if kd % 4 == 3:
                                g = kd // 4
                                a2 = aacc.tile([P, FD], BF16, tag="ea2")
                                nc.vector.tensor_add(a2, pairs[2 * g], pairs[2 * g + 1])
                                ssum_pend.append((g, a2))
                        if ssum_pend and (kc >= n_kc + 3 or (kc >= 7 and (kc - 7) % 4 == 0)):
                            g, a2 = ssum_pend.pop(0)
                            nc.tensor.matmul(ssum_ps, ones_col, a2,
                                             start=(g == 0), stop=(g == 3))
                    rec = asm.tile([1, FD], F32R, tag="rec")
                    with nc.allow_low_precision(reason="f32r rounding of softmax reciprocal"):
                        nc.vector.reciprocal(rec, ssum_ps)
                    recB = ps_s.tile([P, FD], F32, tag="s")
                    nc.tensor.matmul(recB, ones_row, rec, start=True, stop=True)
                    nc.scalar.activation(attnT[:, h, :], pv_ps, AF.Copy)
                    nc.vector.tensor_mul(attnT[:, h, :], attnT[:, h, :], recB)
                # o_proj for this 512-token block
                for sc in range(FD // P):
                    for nt in range(HID // FD):
                        ps = ps_o.tile([P, FD], F32, tag="o")
                        for hh in range(4):
                            nc.tensor.matmul(
                                ps, attnT[:, hh, sc * P:(sc + 1) * P],
                                wo_sb[:, hh, nt * FD:(nt + 1) * FD],
                                start=(hh == 0), stop=(hh == 3))
                        ot = oo.tile([P, FD], F32, tag="ot")
                        nc.vector.tensor_copy(ot, ps)
                        nc.sync.dma_start(
                            out_d[qt * FD + sc * P: qt * FD + (sc + 1) * P,
                                  nt * FD:(nt + 1) * FD], ot)
    nc.compile()
    return nc


# ---------------- host-side packing ----------------

def _pack_core_inputs2(inputs, c, s=S):
    b, hg = c // 4, c % 4
    heads = range(NHL * hg, NHL * hg + NHL)
    f32 = np.float32
    hidden = np.ascontiguousarray(np.asarray(inputs["hidden_states"], dtype=f32)[b, :s])
    mask = np.asarray(inputs["attention_mask"], dtype=f32)[b, 0, 0, :s]
    w_q_down = np.asarray(inputs["w_q_down"], dtype=f32)
    w_kv_down = np.asarray(inputs["w_kv_down"], dtype=f32)
    # fold rms-norm weights into the up-projection weights
    w_q_up = (np.asarray(inputs["w_q_up"], dtype=f32)
              * np.asarray(inputs["q_norm_w"], dtype=f32)[:, None])
    w_kv_up = (np.asarray(inputs["w_kv_up"], dtype=f32)
               * np.asarray(inputs["kv_norm_w"], dtype=f32)[:, None])
    w_o = np.asarray(inputs["w_o"], dtype=f32)
    cos = np.asarray(inputs["cos"], dtype=f32)[:s]
    sin = np.asarray(inputs["sin"], dtype=f32)[:s]

    # w_down layout: q_down | kv_latent_down | k_rope_down duplicated twice
    # (so the roped key lands in both partition halves of KR)
    w_down = np.ascontiguousarray(np.concatenate(
        [w_q_down, w_kv_down[:, :KVLORA],
         w_kv_down[:, KVLORA:], w_kv_down[:, KVLORA:]], axis=1))
    # w_qu layout: 4 nope blocks, then 2 packed rope blocks (heads 0|1, 2|3)
    cols = [w_q_up[:, h * QK: h * QK + NOPE] for h in heads]
    hl = list(heads)
    for h0, h1 in ((hl[0], hl[1]), (hl[2], hl[3])):
        cols.append(np.concatenate(
            [w_q_up[:, h0 * QK + NOPE: (h0 + 1) * QK],
             w_q_up[:, h1 * QK + NOPE: (h1 + 1) * QK]], axis=1))
    w_qu = np.ascontiguousarray(np.concatenate(cols, axis=1))
    w_kvu_kn = np.ascontiguousarray(np.concatenate(
        [w_kv_up[:, h * (NOPE + VD): h * (NOPE + VD) + NOPE] for h in heads], axis=1))
    w_kvu_v = np.ascontiguousarray(np.concatenate(
        [w_kv_up[:, h * (NOPE + VD) + NOPE: (h + 1) * (NOPE + VD)] for h in heads], axis=1))
    w_o_hg = np.ascontiguousarray(np.concatenate(
        [w_o[h * VD: (h + 1) * VD, :] for h in heads], axis=0))

    cosT = cos.T
    sinT = sin.T
    cos2 = np.ascontiguousarray(np.vstack([cosT, cosT]))
    sp64 = np.vstack([sinT[:32], -sinT[:32]])
    sin_pre = np.ascontiguousarray(np.vstack([sp64, sp64]))
    mask_p = np.ascontiguousarray(mask.reshape(s // P, P).T)

    return {
        "xT": np.ascontiguousarray(hidden.T).astype(NPBF16),
        "w_down": w_down.astype(NPBF16),
        "w_qu": w_qu.astype(NPBF16),
        "w_kvu_kn": w_kvu_kn.astype(NPBF16),
        "w_kvu_v": w_kvu_v.astype(NPBF16),
        "w_o": w_o_hg.astype(NPBF16),
        "cos2": cos2,
        "sin_pre": sin_pre,
        "mask_p": mask_p,
        "ones_col": np.ones((P, 1), NPBF16),
        "ones_row": np.ones((1, P), f32),
    }


_NC_CACHE = {}


def kernel(**inputs) -> np.ndarray:
    if "nc" not in _NC_CACHE:
        _NC_CACHE["nc"] = build_nc()
    nc = _NC_CACHE["nc"]
    in_maps = [_pack_core_inputs2(inputs, c) for c in range(8)]
    res = run_bass_kernel_spmd(nc, in_maps, core_ids=list(range(8)))
    outs = [res.results[c]["out"] for c in range(8)]
    full = np.stack([outs[0] + outs[1] + outs[2] + outs[3],
                     outs[4] + outs[5] + outs[6] + outs[7]]).astype(np.float32)
    return full


# revision 4
# speedup vs baseline: 1.0546x; 1.0546x over previous
"""Trainium2 Bass kernel for MultiHeadLatentAttention (MLA) forward — v2.

Sharding: 8 cores = 2 (batch) x 4 (head-groups of 4 heads), as baseline.
v2 changes vs baseline:
  * all matmul operands in bf16 (PE rate unchanged at 1 cycle/row, DMA and
    SBUF footprint halved; fp32 accumulation in PSUM throughout)
  * Q^T / K^T / V / k_rope stay SBUF-resident between the projection and
    attention phases — no DRAM scratch spill/reload
  * rms-norm weights folded into the up-projection weights on the host;
    the norm scale broadcast and the softmax reciprocal broadcast both go
    through tiny ones-matmuls on the PE instead of DRAM round-trips
  * attention runs qt-outer / head-inner and the head-sliced o_proj for
    each 512-token block is emitted right after it, so the output DMA
    overlaps the remaining attention compute
"""
import numpy as np

import concourse.bass as bass
import concourse.tile as tile
from concourse import bacc
from concourse import mybir
from concourse.bass_utils import run_bass_kernel_spmd

# ---- problem dims (hardcoded) ----
B, S, HID = 2, 2048, 2048
NH = 16
QLORA, KVLORA = 1024, 512
NOPE, ROPE, VD = 128, 64, 128
QK = NOPE + ROPE
EPS = 1e-6
NHL = 4               # heads per core
P = 128
FD = 512              # matmul moving/free dim
SCALE = 1.0 / float(np.sqrt(np.float32(QK)))

F32 = mybir.dt.float32
F32R = mybir.dt.float32r
BF16 = mybir.dt.bfloat16
NPBF16 = mybir.dt.np(BF16)
AF = mybir.ActivationFunctionType


def _r(ap):
    return ap.bitcast(F32R)


def build_nc(s=S):
    sq = min(FD, s)
    n_sq = s // sq         # 4 s-quarters for projections
    n_kc = s // P          # 16 attention k chunks
    n_qt = s // FD         # 4 attention q tiles
    nc = bacc.Bacc()

    xT = nc.dram_tensor("xT", [HID, s], BF16, kind="ExternalInput")
    w_down = nc.dram_tensor("w_down", [HID, 1664], BF16, kind="ExternalInput")
    w_qu = nc.dram_tensor("w_qu", [QLORA, 768], BF16, kind="ExternalInput")
    w_kvu_kn = nc.dram_tensor("w_kvu_kn", [KVLORA, 512], BF16, kind="ExternalInput")
    w_kvu_v = nc.dram_tensor("w_kvu_v", [KVLORA, 512], BF16, kind="ExternalInput")
    w_o = nc.dram_tensor("w_o", [512, HID], BF16, kind="ExternalInput")
    cos2_d = nc.dram_tensor("cos2", [P, s], F32, kind="ExternalInput")
    sinp_d = nc.dram_tensor("sin_pre", [P, s], F32, kind="ExternalInput")
    mask_d = nc.dram_tensor("mask_p", [P, n_kc], F32, kind="ExternalInput")
    onesc_d = nc.dram_tensor("ones_col", [P, 1], BF16, kind="ExternalInput")
    onesr_d = nc.dram_tensor("ones_row", [1, P], F32, kind="ExternalInput")

    out_d = nc.dram_tensor("out", [s, HID], F32, kind="ExternalOutput")

    w_down3 = w_down[:, :].rearrange("(hk p) c -> p hk c", p=P)
    xT3 = xT[:, :].rearrange("(hk p) t -> p hk t", p=P)
    w_qu3 = w_qu[:, :].rearrange("(kk p) c -> p kk c", p=P)

    with tile.TileContext(nc) as tc:
      with tc.tile_pool(name="consts", bufs=1) as consts:
        mask_sb = consts.tile([P, n_kc], F32)
        nc.sync.dma_start(mask_sb, mask_d[:, :])
        ones_col = consts.tile([P, 1], BF16)
        nc.sync.dma_start(ones_col, onesc_d[:, :])
        ones_row = consts.tile([1, P], F32R)
        nc.sync.dma_start(ones_row, onesr_d[:, :].bitcast(F32R))
        cos2 = consts.tile([P, s], F32)
        sinp = consts.tile([P, s], F32)
        # SBUF-resident attention operands (written in P, read in A).
        # QT: 4 nope blocks + 2 packed rope blocks (heads 0|1 and 2|3 in
        # partition halves).  KR holds k_rope duplicated in both halves
        # (the host duplicates the rope columns of w_kv_down).
        QT = consts.tile([P, 6, s], BF16)
        KN = consts.tile([P, 4, s], BF16)
        KR = consts.tile([P, s], BF16)
        VT = consts.tile([P, n_kc, 512], BF16)

        # ---------------- P: projections, per s-quarter ----------------
        with tc.tile_pool(name="pw", bufs=1) as pw, \
             tc.tile_pool(name="pwd", bufs=2) as pwd, \
             tc.tile_pool(name="px", bufs=2) as px, \
             tc.tile_pool(name="plat", bufs=2) as plat, \
             tc.tile_pool(name="ptmp", bufs=2) as ptmp, \
             tc.tile_pool(name="pst", bufs=1) as pst, \
             tc.tile_pool(name="pscl", bufs=2) as pscl, \
             tc.tile_pool(name="ps_mm", bufs=4, space="PSUM") as ps_mm, \
             tc.tile_pool(name="ps_sum", bufs=1, space="PSUM") as ps_sum, \
             tc.tile_pool(name="ps_scl", bufs=1, space="PSUM") as ps_scl:
            wqu = pw.tile([P, 8, 768], BF16)
            wkn = pw.tile([P, 4, 512], BF16)
            wv = pw.tile([P, 4, 512], BF16)
            # deferred loads, staggered into quarter 0's rt loop so the first
            # down-proj matmuls aren't gated on these big transfers
            _pw_loads = [
                lambda: nc.sync.dma_start(cos2, cos2_d[:, :]),
                lambda: nc.sync.dma_start(sinp, sinp_d[:, :]),
                lambda: nc.sync.dma_start(
                    wkn, w_kvu_kn[:, :].rearrange("(kk p) c -> p kk c", p=P)),
                lambda: nc.sync.dma_start(
                    wv, w_kvu_v[:, :].rearrange("(kk p) c -> p kk c", p=P)),
                lambda: nc.sync.dma_start(wqu[:, :, 0:256], w_qu3[:, :, 0:256]),
                lambda: nc.sync.dma_start(wqu[:, :, 256:512], w_qu3[:, :, 256:512]),
                lambda: nc.sync.dma_start(wqu[:, :, 512:768], w_qu3[:, :, 512:768]),
            ]

            def rope_from(src, out_t, sl):
                """out = x*cos2 + blockswap(x*sin_pre) on two independent
                64-row halves (packed rope of two heads, or duplicated
                k_rope).

                DVE ops can't read across partitions, so the 32-row block
                swaps go through SBUF->SBUF DMAs into tsw_t.
                """
                tmp_t = ptmp.tile([P, sq], F32, tag="ropetmp")
                tsw_t = ptmp.tile([P, sq], F32, tag="ropesw")
                nc.vector.tensor_mul(tmp_t, src, sinp[:, sl])
                nc.vector.tensor_mul(out_t, src, cos2[:, sl])
                nc.sync.dma_start(tsw_t[0:32], tmp_t[32:64])
                nc.sync.dma_start(tsw_t[32:64], tmp_t[0:32])
                nc.sync.dma_start(tsw_t[64:96], tmp_t[96:128])
                nc.sync.dma_start(tsw_t[96:128], tmp_t[64:96])
                nc.vector.tensor_add(out_t, out_t, tsw_t)

            _xqs = {}

            def get_xq(q):
                if q not in _xqs:
                    xq = px.tile([P, 16, sq], BF16, tag="xq")
                    nc.sync.dma_start(xq, xT3[:, :, q * sq:(q + 1) * sq])
                    _xqs[q] = xq
                return _xqs[q]

            def emit_down_stats(q):
                """Down-proj + rms stats + norm muls for quarter q.

                The sum-of-squares matmul for block rt is emitted after the
                down matmuls of block rt+1 so the PE never head-of-line
                waits on the ACT Square latency.
                """
                sl = slice(q * sq, (q + 1) * sq)
                xq = get_xq(q)
                if q + 1 < n_sq:
                    get_xq(q + 1)  # prefetch next quarter behind this one
                lat = plat.tile([P, 12, sq], BF16, tag="lat")
                sumq_ps = ps_sum.tile([1, sq], F32, tag="sumq")
                sumkv_ps = ps_sum.tile([1, sq], F32, tag="sumkv")
                pend = None
                for rt in range(13):
                    wd_t = pwd.tile([P, 16, P], BF16, tag="wd")
                    nc.sync.dma_start(wd_t, w_down3[:, :, rt * P:(rt + 1) * P])
                    if q == 0 and 2 <= rt < 2 + len(_pw_loads):
                        _pw_loads[rt - 2]()
                    ps = ps_mm.tile([P, sq], F32, tag="dps")
                    for hk in range(16):
                        nc.tensor.matmul(
                            ps, wd_t[:, hk, :], xq[:, hk, :],
                            start=(hk == 0), stop=(hk == 15))
                    if pend is not None:
                        nc.tensor.matmul(*pend)
                        pend = None
                    if rt < 12:
                        sq_t = ptmp.tile([P, sq], BF16, tag="sq")
                        nc.scalar.activation(sq_t, ps, AF.Square)
                        if rt % 2 == 0:
                            nc.vector.tensor_copy(lat[:, rt, :], ps)
                        else:
                            nc.scalar.copy(lat[:, rt, :], ps)
                        tgt = sumq_ps if rt < 8 else sumkv_ps
                        pend = (tgt, ones_col, sq_t,
                                rt in (0, 8), rt in (7, 11))
                    else:
                        rope_from(ps, KR[:, sl], sl)
                # -- rmsnorm scales: rsqrt(sum/n + eps), Newton-refined,
                #    broadcast to 128 partitions via ones-matmul --
                scls = []
                for nm, sums, n_el in (("q", sumq_ps, QLORA), ("kv", sumkv_ps, KVLORA)):
                    m_t = pst.tile([1, sq], F32, tag=f"m{nm}", name=f"m{nm}")
                    nc.vector.tensor_scalar(
                        m_t, sums, 1.0 / n_el, EPS,
                        op0=mybir.AluOpType.mult, op1=mybir.AluOpType.add)
                    inv_t = pst.tile([1, sq], F32, tag=f"inv{nm}", name=f"inv{nm}")
                    nc.vector.reciprocal(inv_t, m_t)
                    y_t = pst.tile([1, sq], F32R, tag=f"y{nm}", name=f"y{nm}")
                    nc.scalar.activation(y_t, inv_t, AF.Sqrt)
                    t_t = pst.tile([1, sq], F32, tag=f"t{nm}", name=f"t{nm}")
                    nc.vector.tensor_mul(t_t, y_t, y_t)
                    nc.vector.tensor_mul(t_t, t_t, m_t)
                    nc.vector.tensor_scalar(
                        t_t, t_t, -0.5, 1.5,
                        op0=mybir.AluOpType.mult, op1=mybir.AluOpType.add)
                    nc.vector.tensor_mul(y_t, y_t, t_t)
                    scl_ps = ps_scl.tile([P, sq], F32, tag="scl")
                    nc.tensor.matmul(scl_ps, ones_row, y_t, start=True, stop=True)
                    scl = pscl.tile([P, sq], F32, tag=f"scl{nm}")
                    nc.scalar.copy(scl, scl_ps)
                    scls.append(scl)
                for rt in range(12):
                    rs = scls[0] if rt < 8 else scls[1]
                    nc.vector.tensor_mul(lat[:, rt, :], lat[:, rt, :], rs)
                return lat

            def emit_up(q, lat):
                """Up projections for quarter q (emitted one quarter late so
                the PE chews quarter q+1's down-proj while q's norm muls
                drain on the DVE)."""
                sl = slice(q * sq, (q + 1) * sq)
                for ro in range(6):
                    ps = ps_mm.tile([P, sq], F32, tag="dps")
                    for kk in range(8):
                        nc.tensor.matmul(
                            ps, wqu[:, kk, ro * P:(ro + 1) * P], lat[:, kk, :],
                            start=(kk == 0), stop=(kk == 7))
                    if ro < 4:
                        nc.any.tensor_copy(QT[:, ro, sl], ps)
                    else:
                        rope_from(ps, QT[:, ro, sl], sl)
                for i in range(4):
                    ps = ps_mm.tile([P, sq], F32, tag="dps")
                    for kk in range(4):
                        nc.tensor.matmul(
                            ps, wkn[:, kk, i * P:(i + 1) * P], lat[:, 8 + kk, :],
                            start=(kk == 0), stop=(kk == 3))
                    nc.any.tensor_copy(KN[:, i, sl], ps)
                for sc in range(sq // P):
                    ps = ps_mm.tile([P, 512], F32, tag="dps")
                    for kk in range(4):
                        nc.tensor.matmul(
                            ps, lat[:, 8 + kk, sc * P:(sc + 1) * P], wv[:, kk, :],
                            start=(kk == 0), stop=(kk == 3))
                    nc.any.tensor_copy(VT[:, q * (sq // P) + sc, :], ps)

            prev = None
            for q in range(n_sq):
                lat = emit_down_stats(q)
                if prev is not None:
                    emit_up(*prev)
                prev = (q, lat)
            emit_up(*prev)

        # ---------------- A + O: attention interleaved with o_proj ----------------
        with tc.tile_pool(name="aw", bufs=1) as aw, \
             tc.tile_pool(name="aat", bufs=2) as aat, \
             tc.tile_pool(name="aexp", bufs=8) as aexp, \
             tc.tile_pool(name="aacc", bufs=2) as aacc, \
             tc.tile_pool(name="asm", bufs=2) as asm, \
             tc.tile_pool(name="oo", bufs=3) as oo, \
             tc.tile_pool(name="ps_s", bufs=2, space="PSUM") as ps_s, \
             tc.tile_pool(name="ps_pv", bufs=2, space="PSUM") as ps_pv, \
             tc.tile_pool(name="ps_ss", bufs=2, space="PSUM") as ps_ss, \
             tc.tile_pool(name="ps_o", bufs=2, space="PSUM") as ps_o:
            wo_sb = aw.tile([P, 4, HID], BF16)
            nc.sync.dma_start(wo_sb, w_o[:, :].rearrange("(c p) n -> p c n", p=P))
            def emit_attn(qt):
                qsl = slice(qt * FD, (qt + 1) * FD)
                attnT = aat.tile([P, 4, FD], BF16, tag="attnT")
                for h in range(4):
                    pv_ps = ps_pv.tile([P, FD], F32, tag="pv")
                    ssum_ps = ps_ss.tile([1, FD], F32, tag="ss")
                    r0 = 64 * (h % 2)
                    rsl = slice(r0, r0 + 64)
                    es = []
                    pairs = {}
                    ssum_pend = []
                    for kc in range(n_kc + 4):
                        if kc < n_kc:
                            ks = slice(kc * P, (kc + 1) * P)
                            s_ps = ps_s.tile([P, FD], F32, tag="s")
                            nc.tensor.matmul(s_ps, KN[:, h, ks], QT[:, h, qsl],
                                             start=True, stop=False)
                            nc.tensor.matmul(s_ps, KR[rsl, ks],
                                             QT[rsl, 4 + h // 2, qsl],
                                             start=False, stop=True)
                            e_t = aexp.tile([P, FD], BF16, tag="e")
                            nc.scalar.activation(e_t, s_ps, AF.Exp,
                                                 bias=mask_sb[:, kc:kc + 1],
                                                 scale=SCALE)
                            es.append(e_t)
                        if 2 <= kc < n_kc + 2:
                            kd = kc - 2
                            nc.tensor.matmul(pv_ps, VT[:, kd, h * P:(h + 1) * P],
                                             es[kd],
                                             start=(kd == 0), stop=(kd == n_kc - 1))
                            if kd % 2 == 1:
                                pr = aacc.tile([P, FD], BF16, tag=f"pr{(kd // 2) % 2}")
                                nc.vector.tensor_add(pr, es[kd - 1], es[kd])
                                pairs[kd // 2] = pr
                            if kd % 4 == 3:
                                g = kd // 4
                                a2 = aacc.tile([P, FD], BF16, tag="ea2")
                                nc.vector.tensor_add(a2, pairs[2 * g], pairs[2 * g + 1])
                                ssum_pend.append((g, a2))
                        if ssum_pend and (kc >= n_kc + 3 or (kc >= 7 and (kc - 7) % 4 == 0)):
                            g, a2 = ssum_pend.pop(0)
                            nc.tensor.matmul(ssum_ps, ones_col, a2,
                                             start=(g == 0), stop=(g == 3))
                    rec = asm.tile([1, FD], F32R, tag="rec")
                    with nc.allow_low_precision(reason="f32r rounding of softmax reciprocal"):
                        nc.vector.reciprocal(rec, ssum_ps)
                    recB = ps_s.tile([P, FD], F32, tag="s")
                    nc.tensor.matmul(recB, ones_row, rec, start=True, stop=True)
                    nc.scalar.activation(attnT[:, h, :], pv_ps, AF.Copy)
                    nc.vector.tensor_mul(attnT[:, h, :], attnT[:, h, :], recB)
                return attnT

            def emit_oproj(qt, attnT):
                # emitted one q-tile late so the PE chews the next tile's
                # scores while this tile's normalize chain drains on the DVE
                for sc in range(FD // P):
                    for nt in range(HID // FD):
                        ps = ps_o.tile([P, FD], F32, tag="o")
                        for hh in range(4):
                            nc.tensor.matmul(
                                ps, attnT[:, hh, sc * P:(sc + 1) * P],
                                wo_sb[:, hh, nt * FD:(nt + 1) * FD],
                                start=(hh == 0), stop=(hh == 3))
                        ot = oo.tile([P, FD], F32, tag="ot")
                        nc.vector.tensor_copy(ot, ps)
                        nc.sync.dma_start(
                            out_d[qt * FD + sc * P: qt * FD + (sc + 1) * P,
                                  nt * FD:(nt + 1) * FD], ot)

            prev_o = None
            for qt in range(n_qt):
                attnT = emit_attn(qt)
                if prev_o is not None:
                    emit_oproj(*prev_o)
                prev_o = (qt, attnT)
            emit_oproj(*prev_o)
    nc.compile()
    return nc


# ---------------- host-side packing ----------------

def _pack_core_inputs2(inputs, c, s=S):
    b, hg = c // 4, c % 4
    heads = range(NHL * hg, NHL * hg + NHL)
    f32 = np.float32
    hidden = np.ascontiguousarray(np.asarray(inputs["hidden_states"], dtype=f32)[b, :s])
    mask = np.asarray(inputs["attention_mask"], dtype=f32)[b, 0, 0, :s]
    w_q_down = np.asarray(inputs["w_q_down"], dtype=f32)
    w_kv_down = np.asarray(inputs["w_kv_down"], dtype=f32)
    # fold rms-norm weights into the up-projection weights
    w_q_up = (np.asarray(inputs["w_q_up"], dtype=f32)
              * np.asarray(inputs["q_norm_w"], dtype=f32)[:, None])
    w_kv_up = (np.asarray(inputs["w_kv_up"], dtype=f32)
               * np.asarray(inputs["kv_norm_w"], dtype=f32)[:, None])
    w_o = np.asarray(inputs["w_o"], dtype=f32)
    cos = np.asarray(inputs["cos"], dtype=f32)[:s]
    sin = np.asarray(inputs["sin"], dtype=f32)[:s]

    # w_down layout: q_down | kv_latent_down | k_rope_down duplicated twice
    # (so the roped key lands in both partition halves of KR)
    w_down = np.ascontiguousarray(np.concatenate(
        [w_q_down, w_kv_down[:, :KVLORA],
         w_kv_down[:, KVLORA:], w_kv_down[:, KVLORA:]], axis=1))
    # w_qu layout: 4 nope blocks, then 2 packed rope blocks (heads 0|1, 2|3)
    cols = [w_q_up[:, h * QK: h * QK + NOPE] for h in heads]
    hl = list(heads)
    for h0, h1 in ((hl[0], hl[1]), (hl[2], hl[3])):
        cols.append(np.concatenate(
            [w_q_up[:, h0 * QK + NOPE: (h0 + 1) * QK],
             w_q_up[:, h1 * QK + NOPE: (h1 + 1) * QK]], axis=1))
    w_qu = np.ascontiguousarray(np.concatenate(cols, axis=1))
    w_kvu_kn = np.ascontiguousarray(np.concatenate(
        [w_kv_up[:, h * (NOPE + VD): h * (NOPE + VD) + NOPE] for h in heads], axis=1))
    w_kvu_v = np.ascontiguousarray(np.concatenate(
        [w_kv_up[:, h * (NOPE + VD) + NOPE: (h + 1) * (NOPE + VD)] for h in heads], axis=1))
    w_o_hg = np.ascontiguousarray(np.concatenate(
        [w_o[h * VD: (h + 1) * VD, :] for h in heads], axis=0))

    cosT = cos.T
    sinT = sin.T
    cos2 = np.ascontiguousarray(np.vstack([cosT, cosT]))
    sp64 = np.vstack([sinT[:32], -sinT[:32]])
    sin_pre = np.ascontiguousarray(np.vstack([sp64, sp64]))
    mask_p = np.ascontiguousarray(mask.reshape(s // P, P).T)

    return {
        "xT": np.ascontiguousarray(hidden.T).astype(NPBF16),
        "w_down": w_down.astype(NPBF16),
        "w_qu": w_qu.astype(NPBF16),
        "w_kvu_kn": w_kvu_kn.astype(NPBF16),
        "w_kvu_v": w_kvu_v.astype(NPBF16),
        "w_o": w_o_hg.astype(NPBF16),
        "cos2": cos2,
        "sin_pre": sin_pre,
        "mask_p": mask_p,
        "ones_col": np.ones((P, 1), NPBF16),
        "ones_row": np.ones((1, P), f32),
    }


_NC_CACHE = {}


def kernel(**inputs) -> np.ndarray:
    if "nc" not in _NC_CACHE:
        _NC_CACHE["nc"] = build_nc()
    nc = _NC_CACHE["nc"]
    in_maps = [_pack_core_inputs2(inputs, c) for c in range(8)]
    res = run_bass_kernel_spmd(nc, in_maps, core_ids=list(range(8)))
    outs = [res.results[c]["out"] for c in range(8)]
    full = np.stack([outs[0] + outs[1] + outs[2] + outs[3],
                     outs[4] + outs[5] + outs[6] + outs[7]]).astype(np.float32)
    return full
